# revision 1
# baseline (speedup 1.0000x reference)
"""CBOW word2vec negative-sampling loss on 8 Trainium2 NeuronCores.

Strategy (data-parallel over batch):
  - batch B=16384 split into 8 shards of 2048 samples (one per core)
  - u_weight/v_weight concatenated host-side into one [200000, 128] table
    (replicated per core); all 21 embedding-row reads per sample
    (10 ctx + 1 pos + 10 neg, v-rows offset by VOCAB) are indirect DMA
    gathers of 128 rows each ([128,1] offset APs — the only offset shape
    this toolchain generates correct descriptors for), 21 per 128-sample
    block
  - per block on-chip: sum ctx rows (DVE reduce), 11 fused dot products
    (scalar_tensor_tensor with accum_out), clip, softplus = Ln(1+Exp(x))
    on ACT with fused free-dim accumulation into the accumulator column
  - per-core partial sums [128, 16] are summed + averaged on host
"""

import numpy as np

VOCAB = 100000
DIM = 128
B = 16384
CTX = 10
NNEG = 10
N_CORES = 8
P = 128
B_SHARD = B // N_CORES          # 2048
NBLK = B_SHARD // P             # 16
K = CTX + 1 + NNEG              # 21 gathered rows per sample


def _split_excess_waits(nc, mybir, max_waits=1):
    """This walrus build rejects instructions carrying more than ~1 sync
    wait (Tile's kernel-tail drain can carry several). Hoist excess waits
    into standalone nops right before the offending instruction — same
    engine, so the in-order stream gives identical semantics."""
    n_split = 0
    for func in nc.m.functions:
        for bb in func.blocks:
            out = []
            changed = False
            for inst in bb.instructions:
                si = inst.sync_info
                if si is not None and len(si.on_wait) > max_waits:
                    waits = list(si.on_wait)
                    for k, w in enumerate(waits[:-max_waits]):
                        nop = mybir.InstNoOp(
                            name=f"wsplit_{inst.name}_{k}", ins=[], outs=[]
                        )
                        nop.engine = inst.engine
                        nop.sync_info = mybir.SyncInfo(on_wait=[w], on_update=[])
                        nc.register_instruction(nop)
                        out.append(nop)
                        n_split += 1
                    inst.sync_info = mybir.SyncInfo(
                        on_wait=waits[-max_waits:], on_update=si.on_update
                    )
                    changed = True
                out.append(inst)
            if changed:
                bb.instructions = out
    return n_split


_PROGRAM_CACHE = {}


def _build_program(gather_bufs=6):
    if gather_bufs in _PROGRAM_CACHE:
        return _PROGRAM_CACHE[gather_bufs]

    import concourse.bass as bass
    import concourse.tile as tile
    import concourse.mybir as mybir

    f32 = mybir.dt.float32
    i32 = mybir.dt.int32
    ND = K - CTX  # 11 dot products per sample (1 pos + 10 neg)

    nc = bass.Bass()
    table = nc.dram_tensor("table", [2 * VOCAB, DIM], f32, kind="ExternalInput")
    idx = nc.dram_tensor("idx", [P, NBLK * K], i32, kind="ExternalInput")
    out = nc.dram_tensor("out", [P, NBLK], f32, kind="ExternalOutput")

    with tile.TileContext(nc) as tc:
        with (
            tc.tile_pool(name="const", bufs=1) as cpool,
            tc.tile_pool(name="gather", bufs=gather_bufs) as gpool,
            tc.tile_pool(name="small", bufs=4) as spool,
            tc.tile_pool(name="scratch", bufs=4) as scpool,
        ):
            idx_t = cpool.tile([P, NBLK * K], i32)
            nc.sync.dma_start(idx_t[:], idx[:])
            acc = cpool.tile([P, NBLK], f32)

            for j in range(NBLK):
                g = gpool.tile([P, K, DIM], f32, tag="g")
                # One [128,1]-offset gather per role: the only offset-AP
                # shape this walrus generates correct descriptors for.
                for k in range(K):
                    nc.gpsimd.indirect_dma_start(
                        out=g[:, k, :],
                        out_offset=None,
                        in_=table[:],
                        in_offset=bass.IndirectOffsetOnAxis(
                            ap=idx_t[:, j * K + k : j * K + k + 1], axis=0
                        ),
                    )

                # sum of the 10 context rows -> [P, DIM]
                su = spool.tile([P, DIM], f32, tag="su")
                nc.vector.tensor_reduce(
                    out=su[:],
                    in_=g[:, 0:CTX, :].rearrange("p n d -> p d n"),
                    axis=mybir.AxisListType.X,
                    op=mybir.AluOpType.add,
                )

                # 11 fused dots: raw[:, n] = sum_d (±0.1 * v_row_n) * su
                # n=0 (pos sample) carries the minus sign so that the loss is
                # softplus(raw_n) uniformly for all n.
                raw = spool.tile([P, ND], f32, tag="raw")
                for n in range(ND):
                    so = scpool.tile([P, DIM], f32, tag="so")
                    nc.vector.scalar_tensor_tensor(
                        out=so[:],
                        in0=g[:, CTX + n, :],
                        scalar=(-1.0 if n == 0 else 1.0) / CTX,
                        in1=su[:],
                        op0=mybir.AluOpType.mult,
                        op1=mybir.AluOpType.mult,
                        accum_out=raw[:, n : n + 1],
                    )

                # clip to [-10, 10] in one fused op
                rc = spool.tile([P, ND], f32, tag="rc")
                nc.vector.tensor_scalar(
                    out=rc[:],
                    in0=raw[:],
                    scalar1=-10.0,
                    scalar2=10.0,
                    op0=mybir.AluOpType.max,
                    op1=mybir.AluOpType.min,
                )

                # softplus(x) = ln(1 + exp(x)); accumulate the 11 terms into
                # this block's accumulator column.
                ex = scpool.tile([P, ND], f32, tag="ex")
                nc.scalar.activation(
                    out=ex[:],
                    in_=rc[:],
                    func=mybir.ActivationFunctionType.Exp,
                )
                sp = scpool.tile([P, ND], f32, tag="sp")
                nc.scalar.activation(
                    out=sp[:],
                    in_=ex[:],
                    func=mybir.ActivationFunctionType.Ln,
                    bias=1.0,
                    accum_out=acc[:, j : j + 1],
                )

            nc.sync.dma_start(out[:], acc[:])

    _split_excess_waits(nc, mybir)
    _PROGRAM_CACHE[gather_bufs] = nc
    return nc


def _prep_inputs(pos_u, pos_v, neg_v, u_weight, v_weight):
    """Shard + repack host-side. Returns per-core input maps."""
    table = np.ascontiguousarray(
        np.concatenate(
            [np.asarray(u_weight, np.float32), np.asarray(v_weight, np.float32)],
            axis=0,
        )
    )
    pos_u = np.asarray(pos_u, np.int32)
    pos_v = np.asarray(pos_v, np.int32)
    neg_v = np.asarray(neg_v, np.int32)

    in_maps = []
    for c in range(N_CORES):
        s = slice(c * B_SHARD, (c + 1) * B_SHARD)
        ia = np.empty((B_SHARD, K), np.int32)
        ia[:, 0:CTX] = pos_u[s]
        ia[:, CTX] = pos_v[s] + VOCAB
        ia[:, CTX + 1 : K] = neg_v[s] + VOCAB
        idx_dram = np.ascontiguousarray(
            ia.reshape(NBLK, P, K).transpose(1, 0, 2).reshape(P, NBLK * K)
        )
        in_maps.append({"table": table, "idx": idx_dram})
    return in_maps


def _run(pos_u, pos_v, neg_v, u_weight, v_weight, trace=False):
    from concourse.bass_utils import run_bass_kernel_spmd

    nc = _build_program()
    in_maps = _prep_inputs(pos_u, pos_v, neg_v, u_weight, v_weight)
    res = run_bass_kernel_spmd(nc, in_maps, list(range(N_CORES)), trace=trace)
    total = 0.0
    for c in range(N_CORES):
        total += res.results[c]["out"].sum(dtype=np.float64)
    loss = np.array(total / B, dtype=np.float32)
    return loss, res


def kernel(pos_u, pos_v, neg_v, u_weight, v_weight):
    loss, _ = _run(pos_u, pos_v, neg_v, u_weight, v_weight, trace=False)
    return loss



# revision 3
# speedup vs baseline: 1.2532x; 1.2532x over previous
"""CBOW word2vec negative-sampling loss on 8 Trainium2 NeuronCores.

Strategy (data-parallel over batch, bulk SWDGE gathers):
  - batch B=16384 split into 8 shards of 2048 samples (one per core)
  - per core the 2048x21 needed embedding rows (10 ctx + 1 pos + 10 neg,
    v-rows offset by VOCAB) are compacted host-side into two per-half
    tables of <=21504 touched rows ("row-shard with all-gather of touched
    rows"); the device gathers all 21504 rows per half with 21 dma_gather
    instructions of 1024 rows each (int16 indices into the compacted
    table; 1024 is the per-instruction ucode limit) instead of 336
    per-block indirect DMAs -- descriptor generation on GpSimd drops from
    ~380us to ~55us and the 16 DMA engines stay fed
  - per block on-chip: sum ctx rows (DVE reduce), 11 fused dot products
    (scalar_tensor_tensor with accum_out), clip, softplus = Ln(1+Exp(x))
    on ACT with fused free-dim accumulation into the accumulator column
  - per-core partial sums [128, 16] are summed + averaged on host
"""

import numpy as np

VOCAB = 100000
DIM = 128
B = 16384
CTX = 10
NNEG = 10
N_CORES = 8
P = 128
B_SHARD = B // N_CORES          # 2048
NBLK = B_SHARD // P             # 16
K = CTX + 1 + NNEG              # 21 gathered rows per sample

N_HALF = 2                      # compacted-table granularity (8 blocks each)
HALF_BLKS = NBLK // N_HALF      # 8
SLOTS_PER_HALF = HALF_BLKS * K  # 168
IDX_PER_HALF = SLOTS_PER_HALF * P   # 21504
CTAB_ROWS = IDX_PER_HALF        # worst-case unique rows per half
G_IDX = 1024                    # rows per dma_gather (ucode per-inst limit)
G_SLOTS = G_IDX // P            # 8 slots per gather
G_PER_HALF = SLOTS_PER_HALF // G_SLOTS  # 21
G_COLS = G_IDX // 16            # 64 idx columns per gather
IDX_COLS = N_HALF * G_PER_HALF * G_COLS  # 2688


def _split_excess_waits(nc, mybir, max_waits=1):
    """This walrus build rejects instructions carrying more than ~1 sync
    wait (Tile's kernel-tail drain can carry several). Hoist excess waits
    into standalone nops right before the offending instruction — same
    engine, so the in-order stream gives identical semantics."""
    n_split = 0
    for func in nc.m.functions:
        for bb in func.blocks:
            out = []
            changed = False
            for inst in bb.instructions:
                si = inst.sync_info
                if si is not None and len(si.on_wait) > max_waits:
                    waits = list(si.on_wait)
                    for k, w in enumerate(waits[:-max_waits]):
                        nop = mybir.InstNoOp(
                            name=f"wsplit_{inst.name}_{k}", ins=[], outs=[]
                        )
                        nop.engine = inst.engine
                        nop.sync_info = mybir.SyncInfo(on_wait=[w], on_update=[])
                        nc.register_instruction(nop)
                        out.append(nop)
                        n_split += 1
                    inst.sync_info = mybir.SyncInfo(
                        on_wait=waits[-max_waits:], on_update=si.on_update
                    )
                    changed = True
                out.append(inst)
            if changed:
                bb.instructions = out
    return n_split


_PROGRAM_CACHE = {}


def _build_program(key=0):
    if key in _PROGRAM_CACHE:
        return _PROGRAM_CACHE[key]

    import concourse.bass as bass
    import concourse.tile as tile
    import concourse.mybir as mybir
    from concourse import library_config
    from concourse.library_overlay import lower_extended_insts

    f32 = mybir.dt.float32
    i16 = mybir.dt.int16
    ND = K - CTX  # 11 dot products per sample (1 pos + 10 neg)

    nc = bass.Bass()
    ctabs = [
        nc.dram_tensor(f"ctab{h}", [CTAB_ROWS, DIM], f32, kind="ExternalInput")
        for h in range(N_HALF)
    ]
    idx = nc.dram_tensor("idx", [P, IDX_COLS], i16, kind="ExternalInput")
    out = nc.dram_tensor("out", [P, NBLK], f32, kind="ExternalOutput")

    with tile.TileContext(nc) as tc:
        with (
            tc.tile_pool(name="const", bufs=1) as cpool,
            tc.tile_pool(name="gather", bufs=2) as gpool,
            tc.tile_pool(name="small", bufs=4) as spool,
            tc.tile_pool(name="scratch", bufs=4) as scpool,
        ):
            nc.gpsimd.load_library(library_config.mlp)
            idx_t = cpool.tile([P, IDX_COLS], i16)
            nc.sync.dma_start(idx_t[:], idx[:])
            acc = cpool.tile([P, NBLK], f32)

            for h in range(N_HALF):
                g = gpool.tile([P, SLOTS_PER_HALF, DIM], f32, tag="g")
                for t in range(G_PER_HALF):
                    col0 = (h * G_PER_HALF + t) * G_COLS
                    nc.gpsimd.dma_gather(
                        out_ap=g[:, t * G_SLOTS : (t + 1) * G_SLOTS, :],
                        in_ap=ctabs[h][:],
                        idxs_ap=idx_t[:, col0 : col0 + G_COLS],
                        num_idxs=G_IDX,
                        num_idxs_reg=G_IDX,
                        elem_size=DIM,
                    )

                for jj in range(HALF_BLKS):
                    j = h * HALF_BLKS + jj
                    base = jj * K

                    # sum of the 10 context rows -> [P, DIM]
                    su = spool.tile([P, DIM], f32, tag="su")
                    nc.vector.tensor_reduce(
                        out=su[:],
                        in_=g[:, base : base + CTX, :].rearrange("p n d -> p d n"),
                        axis=mybir.AxisListType.X,
                        op=mybir.AluOpType.add,
                    )

                    # 11 fused dots: raw[:, n] = sum_d (±1/CTX * v_row_n) * su
                    # n=0 (pos sample) carries the minus sign so the loss is
                    # softplus(raw_n) uniformly for all n.
                    raw = spool.tile([P, ND], f32, tag="raw")
                    for n in range(ND):
                        so = scpool.tile([P, DIM], f32, tag="so")
                        nc.vector.scalar_tensor_tensor(
                            out=so[:],
                            in0=g[:, base + CTX + n, :],
                            scalar=(-1.0 if n == 0 else 1.0) / CTX,
                            in1=su[:],
                            op0=mybir.AluOpType.mult,
                            op1=mybir.AluOpType.mult,
                            accum_out=raw[:, n : n + 1],
                        )

                    # clip to [-10, 10] in one fused op
                    rc = spool.tile([P, ND], f32, tag="rc")
                    nc.vector.tensor_scalar(
                        out=rc[:],
                        in0=raw[:],
                        scalar1=-10.0,
                        scalar2=10.0,
                        op0=mybir.AluOpType.max,
                        op1=mybir.AluOpType.min,
                    )

                    # softplus(x) = ln(1 + exp(x)); accumulate the 11 terms
                    # into this block's accumulator column.
                    ex = scpool.tile([P, ND], f32, tag="ex")
                    nc.scalar.activation(
                        out=ex[:],
                        in_=rc[:],
                        func=mybir.ActivationFunctionType.Exp,
                    )
                    sp = scpool.tile([P, ND], f32, tag="sp")
                    nc.scalar.activation(
                        out=sp[:],
                        in_=ex[:],
                        func=mybir.ActivationFunctionType.Ln,
                        bias=1.0,
                        accum_out=acc[:, j : j + 1],
                    )

            nc.sync.dma_start(out[:], acc[:])

    lower_extended_insts(nc)
    _split_excess_waits(nc, mybir)
    _PROGRAM_CACHE[key] = nc
    return nc


def _prep_inputs(pos_u, pos_v, neg_v, u_weight, v_weight):
    """Shard + compact host-side. Returns per-core input maps."""
    table = np.ascontiguousarray(
        np.concatenate(
            [np.asarray(u_weight, np.float32), np.asarray(v_weight, np.float32)],
            axis=0,
        )
    )
    pos_u = np.asarray(pos_u, np.int64)
    pos_v = np.asarray(pos_v, np.int64)
    neg_v = np.asarray(neg_v, np.int64)

    in_maps = []
    for c in range(N_CORES):
        s = slice(c * B_SHARD, (c + 1) * B_SHARD)
        # ia[b, k]: global row id for sample b, role k  (b within shard)
        ia = np.empty((B_SHARD, K), np.int64)
        ia[:, 0:CTX] = pos_u[s]
        ia[:, CTX] = pos_v[s] + VOCAB
        ia[:, CTX + 1 : K] = neg_v[s] + VOCAB
        # slot-major flat order per half: ids[slot, p] with slot = jj*K + k
        ids = (
            ia.reshape(N_HALF, HALF_BLKS, P, K)
            .transpose(0, 1, 3, 2)
            .reshape(N_HALF, SLOTS_PER_HALF * P)
        )
        m = {}
        idx_np = np.empty((P, IDX_COLS), np.int16)
        for h in range(N_HALF):
            uniq, inv = np.unique(ids[h], return_inverse=True)
            ctab = np.zeros((CTAB_ROWS, DIM), np.float32)
            ctab[: len(uniq)] = table[uniq]
            m[f"ctab{h}"] = ctab
            # gather t covers flat positions [t*G_IDX, (t+1)*G_IDX);
            # idx i of a gather lives at [i % 16, col0 + i // 16]
            cols = inv.astype(np.int16).reshape(G_PER_HALF, G_COLS, 16)
            half_cols = slice(h * G_PER_HALF * G_COLS, (h + 1) * G_PER_HALF * G_COLS)
            idx_np[:16, half_cols] = cols.transpose(2, 0, 1).reshape(
                16, G_PER_HALF * G_COLS
            )
        idx_np[16:, :] = np.tile(idx_np[:16, :], (7, 1))
        m["idx"] = idx_np
        in_maps.append(m)
    return in_maps


def _run(pos_u, pos_v, neg_v, u_weight, v_weight, trace=False):
    from concourse.bass_utils import run_bass_kernel_spmd

    nc = _build_program()
    in_maps = _prep_inputs(pos_u, pos_v, neg_v, u_weight, v_weight)
    res = run_bass_kernel_spmd(nc, in_maps, list(range(N_CORES)), trace=trace)
    total = 0.0
    for c in range(N_CORES):
        total += res.results[c]["out"].sum(dtype=np.float64)
    loss = np.array(total / B, dtype=np.float32)
    return loss, res


def kernel(pos_u, pos_v, neg_v, u_weight, v_weight):
    loss, _ = _run(pos_u, pos_v, neg_v, u_weight, v_weight, trace=False)
    return loss


# revision 6
# speedup vs baseline: 3.0677x; 2.4478x over previous
"""CBOW word2vec negative-sampling loss on 8 Trainium2 NeuronCores.

Strategy (data-parallel over batch, bulk SWDGE gathers on 4 queues):
  - batch B=16384 split into 8 shards of 2048 samples (one per core)
  - per core the 2048x21 needed embedding rows (10 ctx + 1 pos + 10 neg,
    v-rows offset by VOCAB) are compacted host-side into four per-quarter
    tables of <=10752 touched rows ("row-shard with all-gather of touched
    rows"); the device gathers them with 44 dma_gather instructions of
    <=1024 rows each (int16 indices into the compacted tables; 1024 is
    the per-instruction ucode idx-read limit), round-robined over 4 SWDGE
    queues so descriptor generation runs on all 4 GpSimd core pairs in
    parallel instead of serializing on one
  - compute per quarter (4 blocks of 128 samples at once, 6 DVE + 2 ACT
    instructions instead of per-block loops): strided reduce for the ctx
    sums, one broadcast scalar_tensor_tensor + reduce for all 44x11 dot
    products, sign flip via a constant tile, clip, softplus on ACT,
    per-block sums reduced straight into the accumulator
  - per-core partial sums [128, 16] are summed + averaged on host
"""

import numpy as np

VOCAB = 100000
DIM = 128
B = 16384
CTX = 10
NNEG = 10
N_CORES = 8
P = 128
B_SHARD = B // N_CORES          # 2048
NBLK = B_SHARD // P             # 16
K = CTX + 1 + NNEG              # 21 gathered rows per sample
ND = K - CTX                    # 11 dot products per sample

N_GROUP = 4                     # compacted-table granularity (4 blocks each)
GRP_BLKS = NBLK // N_GROUP      # 4
SLOTS_PER_GRP = GRP_BLKS * K    # 84
IDX_PER_GRP = SLOTS_PER_GRP * P  # 10752
CTAB_ROWS = IDX_PER_GRP         # worst-case unique rows per group
G_IDX = 1024                    # max rows per dma_gather (ucode limit)
N_QUEUES = 4                    # SWDGE queues (one GpSimd core pair each)
# per group: 10 gathers of 1024 rows + 1 of 512
GRP_GATHER_SIZES = [G_IDX] * (IDX_PER_GRP // G_IDX) + (
    [IDX_PER_GRP % G_IDX] if IDX_PER_GRP % G_IDX else []
)
GRP_COLS = IDX_PER_GRP // 16    # 672 idx columns per group
IDX_COLS = N_GROUP * GRP_COLS   # 2688


def _split_excess_waits(nc, mybir, max_waits=1):
    """This walrus build rejects instructions carrying more than ~1 sync
    wait (Tile's kernel-tail drain can carry several). Hoist excess waits
    into standalone nops right before the offending instruction — same
    engine, so the in-order stream gives identical semantics."""
    n_split = 0
    for func in nc.m.functions:
        for bb in func.blocks:
            out = []
            changed = False
            for inst in bb.instructions:
                si = inst.sync_info
                if si is not None and len(si.on_wait) > max_waits:
                    waits = list(si.on_wait)
                    for k, w in enumerate(waits[:-max_waits]):
                        nop = mybir.InstNoOp(
                            name=f"wsplit_{inst.name}_{k}", ins=[], outs=[]
                        )
                        nop.engine = inst.engine
                        nop.sync_info = mybir.SyncInfo(on_wait=[w], on_update=[])
                        nc.register_instruction(nop)
                        out.append(nop)
                        n_split += 1
                    inst.sync_info = mybir.SyncInfo(
                        on_wait=waits[-max_waits:], on_update=si.on_update
                    )
                    changed = True
                out.append(inst)
            if changed:
                bb.instructions = out
    return n_split


_PROGRAM_CACHE = {}


def _build_program(key=0):
    if key in _PROGRAM_CACHE:
        return _PROGRAM_CACHE[key]

    import concourse.bass as bass
    import concourse.tile as tile
    import concourse.mybir as mybir
    from concourse import library_config
    from concourse.library_overlay import lower_extended_insts

    f32 = mybir.dt.float32
    i16 = mybir.dt.int16

    nc = bass.Bass(num_swdge_queues=N_QUEUES)
    ctabs = [
        nc.dram_tensor(f"ctab{q}", [CTAB_ROWS, DIM], f32, kind="ExternalInput")
        for q in range(N_GROUP)
    ]
    idx = nc.dram_tensor("idx", [P, IDX_COLS], i16, kind="ExternalInput")
    out = nc.dram_tensor("out", [P, NBLK], f32, kind="ExternalOutput")

    with tile.TileContext(nc) as tc:
        with (
            tc.tile_pool(name="const", bufs=1) as cpool,
            tc.tile_pool(name="gather", bufs=2) as gpool,
            tc.tile_pool(name="dots", bufs=2) as tvpool,
            tc.tile_pool(name="small", bufs=4) as spool,
        ):
            nc.gpsimd.load_library(library_config.mlp)
            idx_t = cpool.tile([P, IDX_COLS], i16)
            nc.sync.dma_start(idx_t[:], idx[:])
            acc = cpool.tile([P, NBLK], f32)
            # sign[:, b, 0] = -1 (positive sample), +1 elsewhere
            sign = cpool.tile([P, GRP_BLKS * ND], f32)
            nc.vector.memset(sign[:], 1.0)
            nc.vector.memset(
                sign[:].rearrange("p (b n) -> p b n", n=ND)[:, :, 0:1], -1.0
            )

            gi = 0  # global gather counter for queue round-robin
            for q in range(N_GROUP):
                g = gpool.tile([P, SLOTS_PER_GRP, DIM], f32, tag="g")
                col0 = q * GRP_COLS
                pos = 0
                for sz in GRP_GATHER_SIZES:
                    cols = sz // 16
                    nc.gpsimd.dma_gather(
                        out_ap=g[:, pos // P : (pos + sz) // P, :],
                        in_ap=ctabs[q][:],
                        idxs_ap=idx_t[:, col0 : col0 + cols],
                        num_idxs=sz,
                        num_idxs_reg=sz,
                        elem_size=DIM,
                        queue_num=gi % N_QUEUES,
                    )
                    col0 += cols
                    pos += sz
                    gi += 1

                # ctx sums + broadcast dot-product multiplies, per block
                # (walrus compute APs are limited to 3 dims)
                su = spool.tile([P, GRP_BLKS, DIM], f32, tag="su")
                tv = tvpool.tile([P, GRP_BLKS * ND, DIM], f32, tag="tv")
                for b in range(GRP_BLKS):
                    base = b * K
                    nc.vector.tensor_reduce(
                        out=su[:, b, :],
                        in_=g[:, base : base + CTX, :].rearrange("p s d -> p d s"),
                        axis=mybir.AxisListType.X,
                        op=mybir.AluOpType.add,
                    )
                    su_b = su[:, b, :][:, None, :].broadcast_to([P, ND, DIM])
                    nc.vector.scalar_tensor_tensor(
                        out=tv[:, b * ND : (b + 1) * ND, :],
                        in0=g[:, base + CTX : base + K, :],
                        scalar=1.0 / CTX,
                        in1=su_b,
                        op0=mybir.AluOpType.mult,
                        op1=mybir.AluOpType.mult,
                    )

                # all 44 dot products: reduce over d -> [P, 44]
                raw = spool.tile([P, GRP_BLKS * ND], f32, tag="raw")
                nc.vector.tensor_reduce(
                    out=raw[:],
                    in_=tv[:],
                    axis=mybir.AxisListType.X,
                    op=mybir.AluOpType.add,
                )

                # sign flip for the positive sample column
                rs = spool.tile([P, GRP_BLKS * ND], f32, tag="rs")
                nc.vector.tensor_tensor(
                    out=rs[:], in0=raw[:], in1=sign[:], op=mybir.AluOpType.mult
                )

                # clip to [-10, 10]
                rc = spool.tile([P, GRP_BLKS * ND], f32, tag="rc")
                nc.vector.tensor_scalar(
                    out=rc[:],
                    in0=rs[:],
                    scalar1=-10.0,
                    scalar2=10.0,
                    op0=mybir.AluOpType.max,
                    op1=mybir.AluOpType.min,
                )

                # softplus(x) = ln(1 + exp(x)) on ACT
                ex = spool.tile([P, GRP_BLKS * ND], f32, tag="ex")
                nc.scalar.activation(
                    out=ex[:], in_=rc[:], func=mybir.ActivationFunctionType.Exp
                )
                sp = spool.tile([P, GRP_BLKS * ND], f32, tag="sp")
                nc.scalar.activation(
                    out=sp[:],
                    in_=ex[:],
                    func=mybir.ActivationFunctionType.Ln,
                    bias=1.0,
                )

                # per-block sums straight into the accumulator columns
                nc.vector.tensor_reduce(
                    out=acc[:, q * GRP_BLKS : (q + 1) * GRP_BLKS],
                    in_=sp[:].rearrange("p (b n) -> p b n", n=ND),
                    axis=mybir.AxisListType.X,
                    op=mybir.AluOpType.add,
                )

            nc.sync.dma_start(out[:], acc[:])

    lower_extended_insts(nc)
    _split_excess_waits(nc, mybir)
    _PROGRAM_CACHE[key] = nc
    return nc


def _prep_inputs(pos_u, pos_v, neg_v, u_weight, v_weight):
    """Shard + compact host-side. Returns per-core input maps."""
    table = np.ascontiguousarray(
        np.concatenate(
            [np.asarray(u_weight, np.float32), np.asarray(v_weight, np.float32)],
            axis=0,
        )
    )
    pos_u = np.asarray(pos_u, np.int64)
    pos_v = np.asarray(pos_v, np.int64)
    neg_v = np.asarray(neg_v, np.int64)

    in_maps = []
    for c in range(N_CORES):
        s = slice(c * B_SHARD, (c + 1) * B_SHARD)
        # ia[b, k]: global row id for sample b, role k  (b within shard)
        ia = np.empty((B_SHARD, K), np.int64)
        ia[:, 0:CTX] = pos_u[s]
        ia[:, CTX] = pos_v[s] + VOCAB
        ia[:, CTX + 1 : K] = neg_v[s] + VOCAB
        # slot-major flat order per group: ids[slot, p] with slot = jj*K + k
        ids = (
            ia.reshape(N_GROUP, GRP_BLKS, P, K)
            .transpose(0, 1, 3, 2)
            .reshape(N_GROUP, SLOTS_PER_GRP * P)
        )
        m = {}
        idx_np = np.empty((P, IDX_COLS), np.int16)
        for q in range(N_GROUP):
            uniq, inv = np.unique(ids[q], return_inverse=True)
            ctab = np.zeros((CTAB_ROWS, DIM), np.float32)
            ctab[: len(uniq)] = table[uniq]
            m[f"ctab{q}"] = ctab
            # within each gather, idx i lives at [i % 16, col0 + i // 16]
            segs = []
            pos = 0
            for sz in GRP_GATHER_SIZES:
                seg = inv[pos : pos + sz].astype(np.int16)
                segs.append(seg.reshape(sz // 16, 16).T)
                pos += sz
            idx_np[:16, q * GRP_COLS : (q + 1) * GRP_COLS] = np.hstack(segs)
        idx_np[16:, :] = np.tile(idx_np[:16, :], (7, 1))
        m["idx"] = idx_np
        in_maps.append(m)
    return in_maps


def _run(pos_u, pos_v, neg_v, u_weight, v_weight, trace=False):
    from concourse.bass_utils import run_bass_kernel_spmd

    nc = _build_program()
    in_maps = _prep_inputs(pos_u, pos_v, neg_v, u_weight, v_weight)
    res = run_bass_kernel_spmd(nc, in_maps, list(range(N_CORES)), trace=trace)
    total = 0.0
    for c in range(N_CORES):
        total += res.results[c]["out"].sum(dtype=np.float64)
    loss = np.array(total / B, dtype=np.float32)
    return loss, res


def kernel(pos_u, pos_v, neg_v, u_weight, v_weight):
    loss, _ = _run(pos_u, pos_v, neg_v, u_weight, v_weight, trace=False)
    return loss


# revision 9
# speedup vs baseline: 3.4883x; 1.1371x over previous
"""CBOW word2vec negative-sampling loss on 8 Trainium2 NeuronCores.

Strategy (data-parallel over batch, bulk SWDGE gathers on 4 queues):
  - batch B=16384 split into 8 shards of 2048 samples (one per core)
  - per core the 2048x21 needed embedding rows (10 ctx + 1 pos + 10 neg,
    v-rows offset by VOCAB) are compacted host-side into two per-half
    bf16 tables of <=21504 touched rows ("row-shard with all-gather of
    touched rows"); the device gathers them with 42 dma_gather
    instructions of 1024 rows each (int16 indices into the compacted
    tables; 1024 is the per-instruction ucode idx-read limit),
    round-robined over 4 SWDGE queues so descriptor generation runs on
    all 4 GpSimd core pairs in parallel instead of serializing on one
  - compute per half (8 blocks of 128 samples at once): per-block strided
    reduce for the bf16 ctx sums + broadcast scalar_tensor_tensor for the
    dot terms, then one f32 reduce for all 88 dot products, sign flip via
    a constant tile, clip, softplus on ACT, per-block sums reduced
    straight into the accumulator
  - per-core partial sums [128, 16] are summed + averaged on host
"""

import numpy as np

VOCAB = 100000
DIM = 128
B = 16384
CTX = 10
NNEG = 10
N_CORES = 8
P = 128
B_SHARD = B // N_CORES          # 2048
NBLK = B_SHARD // P             # 16
K = CTX + 1 + NNEG              # 21 gathered rows per sample
ND = K - CTX                    # 11 dot products per sample

N_GROUP = 2                     # compacted-table granularity (8 blocks each)
GRP_BLKS = NBLK // N_GROUP      # 8
SLOTS_PER_GRP = GRP_BLKS * K    # 168
IDX_PER_GRP = SLOTS_PER_GRP * P  # 21504
CTAB_ROWS = IDX_PER_GRP         # worst-case unique rows per group
G_IDX = 1024                    # max rows per dma_gather (ucode limit)
N_QUEUES = 4                    # SWDGE queues (one GpSimd core pair each)
GRP_GATHER_SIZES = [G_IDX] * (IDX_PER_GRP // G_IDX)  # 21 gathers of 1024
GRP_COLS = IDX_PER_GRP // 16    # 1344 idx columns per group
IDX_COLS = N_GROUP * GRP_COLS   # 2688


def _split_excess_waits(nc, mybir, max_waits=1):
    """This walrus build rejects instructions carrying more than ~1 sync
    wait (Tile's kernel-tail drain can carry several). Hoist excess waits
    into standalone nops right before the offending instruction — same
    engine, so the in-order stream gives identical semantics."""
    n_split = 0
    for func in nc.m.functions:
        for bb in func.blocks:
            out = []
            changed = False
            for inst in bb.instructions:
                si = inst.sync_info
                if si is not None and len(si.on_wait) > max_waits:
                    waits = list(si.on_wait)
                    for k, w in enumerate(waits[:-max_waits]):
                        nop = mybir.InstNoOp(
                            name=f"wsplit_{inst.name}_{k}", ins=[], outs=[]
                        )
                        nop.engine = inst.engine
                        nop.sync_info = mybir.SyncInfo(on_wait=[w], on_update=[])
                        nc.register_instruction(nop)
                        out.append(nop)
                        n_split += 1
                    inst.sync_info = mybir.SyncInfo(
                        on_wait=waits[-max_waits:], on_update=si.on_update
                    )
                    changed = True
                out.append(inst)
            if changed:
                bb.instructions = out
    return n_split


_PROGRAM_CACHE = {}


def _build_program(key=0):
    if key in _PROGRAM_CACHE:
        return _PROGRAM_CACHE[key]

    import concourse.bass as bass
    import concourse.tile as tile
    import concourse.mybir as mybir
    from concourse import library_config
    from concourse.library_overlay import lower_extended_insts

    f32 = mybir.dt.float32
    bf16 = mybir.dt.bfloat16
    i16 = mybir.dt.int16

    nc = bass.Bass(num_swdge_queues=N_QUEUES)
    ctabs = [
        nc.dram_tensor(f"ctab{q}", [CTAB_ROWS, DIM], bf16, kind="ExternalInput")
        for q in range(N_GROUP)
    ]
    idx = nc.dram_tensor("idx", [P, IDX_COLS], i16, kind="ExternalInput")
    out = nc.dram_tensor("out", [P, NBLK], f32, kind="ExternalOutput")

    with tile.TileContext(nc) as tc:
        with (
            tc.tile_pool(name="const", bufs=1) as cpool,
            tc.tile_pool(name="gather", bufs=2) as gpool,
            tc.tile_pool(name="dots", bufs=2) as tvpool,
            tc.tile_pool(name="small", bufs=4) as spool,
        ):
            nc.gpsimd.load_library(library_config.mlp)
            idx_t = cpool.tile([P, IDX_COLS], i16)
            nc.sync.dma_start(idx_t[:], idx[:])
            acc = cpool.tile([P, NBLK], f32)
            # sign[:, b, 0] = -1 (positive sample), +1 elsewhere
            sign = cpool.tile([P, GRP_BLKS * ND], f32)
            nc.vector.memset(sign[:], 1.0)
            nc.vector.memset(
                sign[:].rearrange("p (b n) -> p b n", n=ND)[:, :, 0:1], -1.0
            )

            gi = 0  # global gather counter for queue round-robin
            for q in range(N_GROUP):
                g = gpool.tile([P, SLOTS_PER_GRP, DIM], bf16, tag="g")
                col0 = q * GRP_COLS
                pos = 0
                for sz in GRP_GATHER_SIZES:
                    cols = sz // 16
                    nc.gpsimd.dma_gather(
                        out_ap=g[:, pos // P : (pos + sz) // P, :],
                        in_ap=ctabs[q][:],
                        idxs_ap=idx_t[:, col0 : col0 + cols],
                        num_idxs=sz,
                        num_idxs_reg=sz,
                        elem_size=DIM,
                        queue_num=gi % N_QUEUES,
                    )
                    col0 += cols
                    pos += sz
                    gi += 1

                # ctx sums + broadcast dot-product multiplies, per block
                # (walrus compute APs are limited to 3 dims); bf16 keeps
                # DVE at 2x throughput, final dot accumulation is f32
                su = spool.tile([P, GRP_BLKS, DIM], bf16, tag="su")
                tv = tvpool.tile([P, GRP_BLKS * ND, DIM], bf16, tag="tv")
                with nc.allow_low_precision(reason="bf16 embeddings; loss gate 2e-2"):
                    for b in range(GRP_BLKS):
                        base = b * K
                        nc.vector.tensor_reduce(
                            out=su[:, b, :],
                            in_=g[:, base : base + CTX, :].rearrange(
                                "p s d -> p d s"
                            ),
                            axis=mybir.AxisListType.X,
                            op=mybir.AluOpType.add,
                        )
                        su_b = su[:, b, :][:, None, :].broadcast_to([P, ND, DIM])
                        nc.vector.scalar_tensor_tensor(
                            out=tv[:, b * ND : (b + 1) * ND, :],
                            in0=g[:, base + CTX : base + K, :],
                            scalar=1.0 / CTX,
                            in1=su_b,
                            op0=mybir.AluOpType.mult,
                            op1=mybir.AluOpType.mult,
                        )

                # all 44 dot products: reduce over d -> [P, 44]
                raw = spool.tile([P, GRP_BLKS * ND], f32, tag="raw")
                nc.vector.tensor_reduce(
                    out=raw[:],
                    in_=tv[:],
                    axis=mybir.AxisListType.X,
                    op=mybir.AluOpType.add,
                )

                # sign flip for the positive sample column
                rs = spool.tile([P, GRP_BLKS * ND], f32, tag="rs")
                nc.vector.tensor_tensor(
                    out=rs[:], in0=raw[:], in1=sign[:], op=mybir.AluOpType.mult
                )

                # clip to [-10, 10]
                rc = spool.tile([P, GRP_BLKS * ND], f32, tag="rc")
                nc.vector.tensor_scalar(
                    out=rc[:],
                    in0=rs[:],
                    scalar1=-10.0,
                    scalar2=10.0,
                    op0=mybir.AluOpType.max,
                    op1=mybir.AluOpType.min,
                )

                # softplus(x) = ln(1 + exp(x)) on ACT
                ex = spool.tile([P, GRP_BLKS * ND], f32, tag="ex")
                nc.scalar.activation(
                    out=ex[:], in_=rc[:], func=mybir.ActivationFunctionType.Exp
                )
                sp = spool.tile([P, GRP_BLKS * ND], f32, tag="sp")
                nc.scalar.activation(
                    out=sp[:],
                    in_=ex[:],
                    func=mybir.ActivationFunctionType.Ln,
                    bias=1.0,
                )

                # per-block sums straight into the accumulator columns
                nc.vector.tensor_reduce(
                    out=acc[:, q * GRP_BLKS : (q + 1) * GRP_BLKS],
                    in_=sp[:].rearrange("p (b n) -> p b n", n=ND),
                    axis=mybir.AxisListType.X,
                    op=mybir.AluOpType.add,
                )

            nc.sync.dma_start(out[:], acc[:])

    lower_extended_insts(nc)
    _split_excess_waits(nc, mybir)
    _PROGRAM_CACHE[key] = nc
    return nc


def _prep_inputs(pos_u, pos_v, neg_v, u_weight, v_weight):
    """Shard + compact host-side. Returns per-core input maps."""
    import ml_dtypes

    table = np.ascontiguousarray(
        np.concatenate(
            [np.asarray(u_weight, np.float32), np.asarray(v_weight, np.float32)],
            axis=0,
        ).astype(ml_dtypes.bfloat16)
    )
    pos_u = np.asarray(pos_u, np.int64)
    pos_v = np.asarray(pos_v, np.int64)
    neg_v = np.asarray(neg_v, np.int64)

    in_maps = []
    for c in range(N_CORES):
        s = slice(c * B_SHARD, (c + 1) * B_SHARD)
        # ia[b, k]: global row id for sample b, role k  (b within shard)
        ia = np.empty((B_SHARD, K), np.int64)
        ia[:, 0:CTX] = pos_u[s]
        ia[:, CTX] = pos_v[s] + VOCAB
        ia[:, CTX + 1 : K] = neg_v[s] + VOCAB
        # slot-major flat order per group: ids[slot, p] with slot = jj*K + k
        ids = (
            ia.reshape(N_GROUP, GRP_BLKS, P, K)
            .transpose(0, 1, 3, 2)
            .reshape(N_GROUP, SLOTS_PER_GRP * P)
        )
        m = {}
        idx_np = np.empty((P, IDX_COLS), np.int16)
        for q in range(N_GROUP):
            uniq, inv = np.unique(ids[q], return_inverse=True)
            ctab = np.zeros((CTAB_ROWS, DIM), ml_dtypes.bfloat16)
            ctab[: len(uniq)] = table[uniq]
            m[f"ctab{q}"] = ctab
            # within each gather, idx i lives at [i % 16, col0 + i // 16]
            segs = []
            pos = 0
            for sz in GRP_GATHER_SIZES:
                seg = inv[pos : pos + sz].astype(np.int16)
                segs.append(seg.reshape(sz // 16, 16).T)
                pos += sz
            idx_np[:16, q * GRP_COLS : (q + 1) * GRP_COLS] = np.hstack(segs)
        idx_np[16:, :] = np.tile(idx_np[:16, :], (7, 1))
        m["idx"] = idx_np
        in_maps.append(m)
    return in_maps


def _run(pos_u, pos_v, neg_v, u_weight, v_weight, trace=False):
    from concourse.bass_utils import run_bass_kernel_spmd

    nc = _build_program()
    in_maps = _prep_inputs(pos_u, pos_v, neg_v, u_weight, v_weight)
    res = run_bass_kernel_spmd(nc, in_maps, list(range(N_CORES)), trace=trace)
    total = 0.0
    for c in range(N_CORES):
        total += res.results[c]["out"].sum(dtype=np.float64)
    loss = np.array(total / B, dtype=np.float32)
    return loss, res


def kernel(pos_u, pos_v, neg_v, u_weight, v_weight):
    loss, _ = _run(pos_u, pos_v, neg_v, u_weight, v_weight, trace=False)
    return loss
